# revision 12
# baseline (speedup 1.0000x reference)
"""MoE transformer layer on 8 Trainium2 NeuronCores.

Strategy:
  Launch 1 (attention block): shard by (batch, seq-half) -> 8 cores.
    Each core holds all 1024 tokens of its batch (for K/V) with its own
    512 query tokens ordered first, computes LN1 -> MHA -> residual ->
    LN2 entirely in a transposed [E, token] layout (E on partitions, so
    every bias/LN-gain is a per-partition scalar and no transposes are
    needed anywhere). Outputs x2T and h2T per core.
  Host: top-2 gating (softmax over 8 logits, renormalized), builds the
    per-expert token batches (all-to-all dispatch done on host).
  Launch 2 (expert FFN): expert-parallel, core e owns expert e.
    toksT [E, C] -> gelu(w1.T @ toks + b1) -> w2.T @ h + b2 -> outT.
  Host: scatter-add combine with gate weights + residual.
"""

import numpy as np

import concourse.bass as bass
import concourse.tile as tile
from concourse import bacc, mybir
from concourse.bass_utils import run_bass_kernel_spmd

S, B, E = 1024, 4, 1024
H, DH = 16, 64
F, NE = 4096, 8
N = S * B
NCORES = 8
Q = 512          # query tokens per core
KV = 1024        # key/value tokens per core (full batch-b sequence)
C = 1280         # expert capacity (max expert load for seed-0 inputs is 1076)
CT = [(0, 512), (512, 512), (1024, 256)]  # (offset, width) token tiles in launch 2
ET = E // 128    # 8
FT = F // 128    # 32

f32 = mybir.dt.float32
f32r = mybir.dt.float32r
bf16 = mybir.dt.bfloat16
AF = mybir.ActivationFunctionType
ALU = mybir.AluOpType

_GELU = AF.Gelu  # patchable for CoreSim (which lacks Gelu)

_programs = {}


def _bcast_dram(ap2d, nparts):
    """Partition-broadcast DMA source: read a [D,1] dram slice into [nparts, D]."""
    return bass.AP(tensor=ap2d.tensor, offset=ap2d.offset, ap=[[0, nparts]] + ap2d.ap)


def _build_launch1():
    nc = bacc.Bacc("TRN2", target_bir_lowering=False, debug=False, num_devices=NCORES)

    xT_d = nc.dram_tensor("xT", [E, KV], f32, kind="ExternalInput").ap()
    wqkvT_d = nc.dram_tensor("wqkvT", [E, 3 * E], f32, kind="ExternalInput").ap()
    bqkv_d = nc.dram_tensor("bqkv", [3 * E, 1], f32, kind="ExternalInput").ap()
    woT_d = nc.dram_tensor("woT", [E, E], f32, kind="ExternalInput").ap()
    bo_d = nc.dram_tensor("bo", [E, 1], f32, kind="ExternalInput").ap()
    g1_d = nc.dram_tensor("g1", [E, 1], f32, kind="ExternalInput").ap()
    b1_d = nc.dram_tensor("b1", [E, 1], f32, kind="ExternalInput").ap()
    g2_d = nc.dram_tensor("g2", [E, 1], f32, kind="ExternalInput").ap()
    b2_d = nc.dram_tensor("b2", [E, 1], f32, kind="ExternalInput").ap()
    x2T_d = nc.dram_tensor("x2T", [E, Q], f32, kind="ExternalOutput").ap()
    h2T_d = nc.dram_tensor("h2T", [E, Q], f32, kind="ExternalOutput").ap()

    tc_ctx = tile.TileContext(nc)
    with tc_ctx as tc:
        consts = tc.alloc_tile_pool(name="consts", bufs=1)
        statp = tc.alloc_tile_pool(name="stat", bufs=1)
        bcp = tc.alloc_tile_pool(name="bc", bufs=1)
        sqp = tc.alloc_tile_pool(name="sqp", bufs=2)
        wsp = tc.alloc_tile_pool(name="wstream", bufs=4)
        otp = tc.alloc_tile_pool(name="otp", bufs=1)
        outp = tc.alloc_tile_pool(name="outp", bufs=1)
        pmm = tc.alloc_tile_pool(name="pmm", bufs=4, space="PSUM")
        pav = tc.alloc_tile_pool(name="pav", bufs=2, space="PSUM")
        pst = tc.alloc_tile_pool(name="pst", bufs=2, space="PSUM")

        ones128 = consts.tile([128, 1], f32r, tag="ones128")
        nc.vector.memset(ones128[:].bitcast(f32), 1.0)
        ones1 = consts.tile([1, 128], f32r, tag="ones1")
        nc.vector.memset(ones1[:].bitcast(f32), 1.0)
        eps = consts.tile([1, 1], f32, tag="eps")
        nc.vector.memset(eps[:], 1e-5)

        def ppar(dram, k, tag):
            t = consts.tile([128, k], f32, tag=tag, name=tag)
            nc.sync.dma_start(out=t[:], in_=dram.rearrange("(a p) o -> p (a o)", p=128))
            return t

        g1_sb = ppar(g1_d, ET, "g1c")
        b1_sb = ppar(b1_d, ET, "b1c")
        g2_sb = ppar(g2_d, ET, "g2c")
        b2_sb = ppar(b2_d, ET, "b2c")
        bo_sb = ppar(bo_d, ET, "boc")
        bqkv_sb = ppar(bqkv_d, 24, "bqkvc")

        # ---------- LN helper: stats along partitions via ones-matmul ----------
        def ln_stats(src_tiles, ncols, tagpfx):
            s1 = statp.tile([1, KV], f32r, tag="s1row", name=f"{tagpfx}_s1")
            s2 = statp.tile([1, KV], f32r, tag="s2row", name=f"{tagpfx}_s2")
            tmp = statp.tile([1, KV], f32r, tag="tmprow", name=f"{tagpfx}_tmp")
            for h in range(ncols // 512):
                cs = slice(h * 512, (h + 1) * 512)
                p1 = pst.tile([1, 512], f32, tag="st", name=f"{tagpfx}_p1_{h}")
                for i in range(ET):
                    nc.tensor.matmul(p1[:], ones128[:],
                                     src_tiles[i][:, cs],
                                     start=(i == 0), stop=(i == ET - 1))
                nc.vector.tensor_copy(out=s1[:, cs], in_=p1[:])
                p2 = pst.tile([1, 512], f32, tag="st", name=f"{tagpfx}_p2_{h}")
                for i in range(ET):
                    sq = sqp.tile([128, 512], f32r, tag="sq", name=f"{tagpfx}_sq_{h}_{i}")
                    nc.vector.tensor_mul(sq[:], src_tiles[i][:, cs], src_tiles[i][:, cs])
                    nc.tensor.matmul(p2[:], ones128[:], sq[:],
                                     start=(i == 0), stop=(i == ET - 1))
                nc.vector.tensor_copy(out=s2[:, cs], in_=p2[:])
            cs = slice(0, ncols)
            # s1 <- mean ; s2 <- E[x^2] ; tmp <- mean^2 ; s2 <- var
            nc.vector.tensor_scalar(out=s1[:, cs], in0=s1[:, cs], scalar1=1.0 / E,
                                    scalar2=None, op0=ALU.mult)
            nc.vector.tensor_scalar(out=s2[:, cs], in0=s2[:, cs], scalar1=1.0 / E,
                                    scalar2=None, op0=ALU.mult)
            nc.vector.tensor_mul(tmp[:, cs], s1[:, cs], s1[:, cs])
            nc.vector.tensor_sub(s2[:, cs], s2[:, cs], tmp[:, cs])
            # s2 <- rstd = exp(-0.5*ln(var+eps))
            nc.scalar.activation(out=tmp[:, cs], in_=s2[:, cs], func=AF.Ln,
                                 bias=eps[:], scale=1.0)
            nc.scalar.activation(out=s2[:, cs], in_=tmp[:, cs], func=AF.Exp, scale=-0.5)
            # tmp <- beta = -mean*rstd
            nc.vector.tensor_mul(tmp[:, cs], s1[:, cs], s2[:, cs])
            nc.vector.tensor_scalar(out=tmp[:, cs], in0=tmp[:, cs], scalar1=-1.0,
                                    scalar2=None, op0=ALU.mult)
            return s2, tmp

        def bcast_rows(rowap, ncols, tagname):
            dst = bcp.tile([128, ncols], f32, tag=tagname, name=f"bc_{tagname}")
            for h in range(ncols // 512):
                cs = slice(h * 512, (h + 1) * 512)
                pb = pmm.tile([128, 512], f32, tag="mm", name=f"bc_{tagname}_{h}")
                nc.tensor.matmul(pb[:], ones1[:], rowap[:, cs],
                                 start=True, stop=True)
                nc.vector.tensor_copy(out=dst[:, cs], in_=pb[:])
            return dst

        # ---------- phase 1: load x, LN1 ----------
        xqp = tc.alloc_tile_pool(name="xqp", bufs=1)
        lxp = tc.alloc_tile_pool(name="lxp", bufs=1)
        xp = tc.alloc_tile_pool(name="xp", bufs=1)

        x_sb = []
        for i in range(ET):
            t = xp.tile([128, KV], f32r, tag=f"x{i}", name=f"x_sb{i}")
            nc.sync.dma_start(out=t[:], in_=xT_d[i * 128:(i + 1) * 128, :].bitcast(f32r))
            x_sb.append(t)

        rstd1, beta1 = ln_stats(x_sb, KV, "ln1")
        aB1 = bcast_rows(rstd1, KV, "aB1")
        bB1 = bcast_rows(beta1, KV, "bB1")

        lx = []
        xq = []
        for i in range(ET):
            t = lxp.tile([128, KV], f32r, tag=f"lx{i}", name=f"lx{i}")
            nc.vector.tensor_mul(t[:], x_sb[i][:], aB1[:])
            nc.vector.tensor_add(t[:], t[:], bB1[:])
            nc.scalar.activation(out=t[:], in_=t[:], func=AF.Identity,
                                 scale=g1_sb[:, i:i + 1], bias=b1_sb[:, i:i + 1])
            lx.append(t)
            tq = xqp.tile([128, Q], f32, tag=f"xq{i}", name=f"xq{i}")
            nc.vector.tensor_copy(out=tq[:], in_=x_sb[i][:, 0:Q])
            xq.append(tq)
        xp.release()

        # ---------- phase 2: attention ----------
        wvp = tc.alloc_tile_pool(name="wvp", bufs=1)
        vp = tc.alloc_tile_pool(name="vp", bufs=1)
        qkp = tc.alloc_tile_pool(name="qkp", bufs=2)
        attnp = tc.alloc_tile_pool(name="attnp", bufs=3)

        oT = []
        for i in range(ET):
            oT.append(otp.tile([128, Q], f32r, tag=f"oT{i}", name=f"oT{i}"))

        for half in range(2):
            # V projection for this half (8 heads), token-major with ones column
            wv = []
            for kt in range(ET):
                wt = wvp.tile([128, 512], f32r, tag=f"wv{kt}", name=f"wv_{half}_{kt}")
                nc.sync.dma_start(
                    out=wt[:],
                    in_=wqkvT_d[kt * 128:(kt + 1) * 128,
                                2 * E + half * 512: 2 * E + (half + 1) * 512].bitcast(f32r))
                wv.append(wt)
            bvB = bcp.tile([128, 512], f32, tag="bvB", name=f"bvB_{half}")
            nc.sync.dma_start(
                out=bvB[:],
                in_=_bcast_dram(bqkv_d[2 * E + half * 512: 2 * E + (half + 1) * 512, :], 128))
            v_sb = []
            for tt in range(ET):
                pv = pmm.tile([128, 512], f32, tag="mm", name=f"pv_{half}_{tt}")
                for kt in range(ET):
                    nc.tensor.matmul(pv[:],
                                     lx[kt][:, tt * 128:(tt + 1) * 128],
                                     wv[kt][:],
                                     start=(kt == 0), stop=(kt == ET - 1))
                vt = vp.tile([128, 8 * 65], f32r, tag=f"v{tt}", name=f"v_{half}_{tt}")
                nc.vector.tensor_add(
                    vt[:].rearrange("p (h d) -> p h d", h=8)[:, :, 0:64],
                    pv[:].rearrange("p (h d) -> p h d", h=8),
                    bvB[:].rearrange("p (h d) -> p h d", h=8))
                nc.vector.memset(vt[:].rearrange("p (h d) -> p h d", h=8)[:, :, 64:65].bitcast(f32), 1.0)
                v_sb.append(vt)

            for dt in range(half * 4, half * 4 + 4):
                # qT [128, Q] for d-rows dt*128..
                pq = pmm.tile([128, Q], f32, tag="mm", name=f"pq_{dt}")
                for kt in range(ET):
                    wt = wsp.tile([128, 128], f32r, tag="wq", name=f"wq_{dt}_{kt}")
                    nc.sync.dma_start(
                        out=wt[:],
                        in_=wqkvT_d[kt * 128:(kt + 1) * 128, dt * 128:(dt + 1) * 128].bitcast(f32r))
                    nc.tensor.matmul(pq[:], wt[:],
                                     lx[kt][:, 0:Q],
                                     start=(kt == 0), stop=(kt == ET - 1))
                qT = qkp.tile([128, Q], f32r, tag="qT", name=f"qT_{dt}")
                nc.scalar.activation(out=qT[:], in_=pq[:], func=AF.Identity,
                                     bias=bqkv_sb[:, dt:dt + 1], scale=1.0)
                # kT [128, KV]
                kT = qkp.tile([128, KV], f32r, tag="kT", name=f"kT_{dt}")
                pk = [pmm.tile([128, 512], f32, tag="mm", name=f"pk_{dt}_{h}")
                      for h in range(2)]
                for kt in range(ET):
                    wt = wsp.tile([128, 128], f32r, tag="wk", name=f"wk_{dt}_{kt}")
                    nc.sync.dma_start(
                        out=wt[:],
                        in_=wqkvT_d[kt * 128:(kt + 1) * 128,
                                    E + dt * 128: E + (dt + 1) * 128].bitcast(f32r))
                    for h in range(2):
                        nc.tensor.matmul(pk[h][:], wt[:],
                                         lx[kt][:, h * 512:(h + 1) * 512],
                                         start=(kt == 0), stop=(kt == ET - 1))
                for h in range(2):
                    nc.scalar.activation(out=kT[:, h * 512:(h + 1) * 512], in_=pk[h][:],
                                         func=AF.Identity, bias=bqkv_sb[:, 8 + dt:9 + dt],
                                         scale=1.0)

                for hh in range(2):
                    hsub = slice(hh * 64, hh * 64 + 64)
                    hloc = (dt - half * 4) * 2 + hh
                    pav_t = pav.tile([65, Q], f32, tag="av", name=f"pav_{dt}_{hh}")
                    for tt in range(ET):
                        psc = pmm.tile([128, Q], f32, tag="mm", name=f"psc_{dt}_{hh}_{tt}")
                        nc.tensor.matmul(psc[:],
                                         kT[hsub, tt * 128:(tt + 1) * 128],
                                         qT[hsub, :],
                                         start=True, stop=True)
                        at = attnp.tile([128, Q], f32r, tag="attn",
                                        name=f"attn_{dt}_{hh}_{tt}")
                        nc.scalar.activation(out=at[:], in_=psc[:], func=AF.Exp,
                                             scale=0.125)
                        nc.tensor.matmul(
                            pav_t[:],
                            v_sb[tt][:].rearrange("p (h d) -> p h d", h=8)[:, hloc, :],
                            at[:],
                            start=(tt == 0), stop=(tt == ET - 1))
                    rrow = statp.tile([1, Q], f32r, tag="rrow2", name=f"r_{dt}_{hh}",
                                      bufs=2)
                    with nc.allow_low_precision(reason="f32r keeps fp32 bit layout"):
                        nc.vector.reciprocal(rrow[:], pav_t[64:65, :])
                    prb = pmm.tile([128, Q], f32, tag="mm", name=f"prb_{dt}_{hh}")
                    nc.tensor.matmul(prb[0:64, :], ones1[:, 0:64],
                                     rrow[:], start=True, stop=True)
                    rB = attnp.tile([64, Q], f32, tag="rB", name=f"rB_{dt}_{hh}", bufs=2)
                    nc.vector.tensor_copy(out=rB[:], in_=prb[0:64, :])
                    nc.vector.tensor_mul(oT[dt][hsub, :], pav_t[0:64, :], rB[:])

        attnp.release()
        qkp.release()
        vp.release()
        wvp.release()
        lxp.release()

        # ---------- phase 3: out projection + residual -> x2T ----------
        x2 = []
        for et in range(ET):
            po = pmm.tile([128, Q], f32, tag="mm", name=f"po_{et}")
            for dt in range(ET):
                wt = wsp.tile([128, 128], f32r, tag="wo", name=f"wo_{et}_{dt}")
                nc.sync.dma_start(
                    out=wt[:],
                    in_=woT_d[dt * 128:(dt + 1) * 128, et * 128:(et + 1) * 128].bitcast(f32r))
                nc.tensor.matmul(po[:], wt[:], oT[dt][:],
                                 start=(dt == 0), stop=(dt == ET - 1))
            xt = outp.tile([128, Q], f32r, tag=f"x2_{et}", name=f"x2_{et}")
            nc.scalar.activation(out=xt[:], in_=po[:], func=AF.Identity,
                                 bias=bo_sb[:, et:et + 1], scale=1.0)
            nc.vector.tensor_add(xt[:], xt[:], xq[et][:])
            nc.sync.dma_start(out=x2T_d[et * 128:(et + 1) * 128, :], in_=xt[:].bitcast(f32))
            x2.append(xt)

        # ---------- phase 4: LN2 -> h2T ----------
        rstd2, beta2 = ln_stats(x2, Q, "ln2")
        aB2 = bcast_rows(rstd2, Q, "aB2")
        bB2 = bcast_rows(beta2, Q, "bB2")
        for et in range(ET):
            t = outp.tile([128, Q], f32, tag="h2", name=f"h2_{et}", bufs=2)
            nc.vector.tensor_mul(t[:], x2[et][:], aB2[:])
            nc.vector.tensor_add(t[:], t[:], bB2[:])
            nc.scalar.activation(out=t[:], in_=t[:], func=AF.Identity,
                                 scale=g2_sb[:, et:et + 1], bias=b2_sb[:, et:et + 1])
            nc.gpsimd.dma_start(out=h2T_d[et * 128:(et + 1) * 128, :], in_=t[:])

        # release remaining pools in LIFO order per space
        xqp.release()
        outp.release()
        otp.release()
        wsp.release()
        sqp.release()
        bcp.release()
        statp.release()
        consts.release()
        pst.release()
        pav.release()
        pmm.release()

    nc.compile()
    return nc


def _build_launch2():
    nc = bacc.Bacc("TRN2", target_bir_lowering=False, debug=False, num_devices=NCORES)

    toksT_d = nc.dram_tensor("toksT", [E, C], f32, kind="ExternalInput").ap()
    w1_d = nc.dram_tensor("w1", [E, F], f32, kind="ExternalInput").ap()
    w2_d = nc.dram_tensor("w2", [F, E], bf16, kind="ExternalInput").ap()
    b1_d = nc.dram_tensor("b1", [F, 1], f32, kind="ExternalInput").ap()
    b2_d = nc.dram_tensor("b2", [E, 1], f32, kind="ExternalInput").ap()
    outT_d = nc.dram_tensor("outT", [E, C], f32, kind="ExternalOutput").ap()

    with tile.TileContext(nc) as tc:
        with (
            tc.tile_pool(name="consts", bufs=1) as consts,
            tc.tile_pool(name="tok", bufs=1) as tokp,
            tc.tile_pool(name="hp", bufs=1) as hp,
            tc.tile_pool(name="ws", bufs=6) as wsp,
            tc.tile_pool(name="outs", bufs=3) as outs,
            tc.tile_pool(name="pg1", bufs=4, space="PSUM") as pg1,
            tc.tile_pool(name="pg2", bufs=4, space="PSUM") as pg2,
        ):
            b1_sb = consts.tile([128, FT], f32, tag="b1")
            nc.sync.dma_start(out=b1_sb[:], in_=b1_d.rearrange("(a p) o -> p (a o)", p=128))
            b2_sb = consts.tile([128, ET], f32, tag="b2")
            nc.sync.dma_start(out=b2_sb[:], in_=b2_d.rearrange("(a p) o -> p (a o)", p=128))

            toks = []
            for i in range(ET):
                t = tokp.tile([128, C], f32r, tag=f"t{i}", name=f"toks{i}")
                nc.sync.dma_start(out=t[:], in_=toksT_d[i * 128:(i + 1) * 128, :].bitcast(f32r))
                toks.append(t)

            hbf = []
            for ft in range(FT):
                hbf.append(hp.tile([128, C], bf16, tag=f"h{ft}", name=f"hbf{ft}"))

            # GEMM1: hT = gelu(w1.T @ toksT + b1)
            for ft in range(FT):
                ps = [pg1.tile([128, w], f32, tag="g1", name=f"pg1_{ft}_{ci}")
                      for ci, (off, w) in enumerate(CT)]
                for kt in range(ET):
                    wt = wsp.tile([128, 128], f32r, tag="w1", name=f"w1_{ft}_{kt}")
                    nc.sync.dma_start(
                        out=wt[:],
                        in_=w1_d[kt * 128:(kt + 1) * 128, ft * 128:(ft + 1) * 128].bitcast(f32r))
                    for ci, (off, w) in enumerate(CT):
                        nc.tensor.matmul(ps[ci][:], wt[:],
                                         toks[kt][:, off:off + w],
                                         start=(kt == 0), stop=(kt == ET - 1))
                for ci, (off, w) in enumerate(CT):
                    nc.scalar.activation(out=hbf[ft][:, off:off + w], in_=ps[ci][:],
                                         func=_GELU, bias=b1_sb[:, ft:ft + 1], scale=1.0)

            # GEMM2: outT = w2.T @ hT + b2
            for et in range(ET):
                ps = [pg2.tile([128, w], f32, tag="g2", name=f"pg2_{et}_{ci}")
                      for ci, (off, w) in enumerate(CT)]
                for ft in range(FT):
                    wt = wsp.tile([128, 128], bf16, tag="w2", name=f"w2_{et}_{ft}")
                    nc.sync.dma_start(
                        out=wt[:],
                        in_=w2_d[ft * 128:(ft + 1) * 128, et * 128:(et + 1) * 128])
                    for ci, (off, w) in enumerate(CT):
                        nc.tensor.matmul(ps[ci][:], wt[:], hbf[ft][:, off:off + w],
                                         start=(ft == 0), stop=(ft == FT - 1))
                for ci, (off, w) in enumerate(CT):
                    ot = outs.tile([128, 512], f32, tag="ot", name=f"ot_{et}_{ci}")
                    nc.vector.tensor_scalar(out=ot[:, 0:w], in0=ps[ci][:],
                                            scalar1=b2_sb[:, et:et + 1], scalar2=None,
                                            op0=ALU.add)
                    nc.gpsimd.dma_start(out=outT_d[et * 128:(et + 1) * 128, off:off + w],
                                        in_=ot[:, 0:w])

    nc.compile()
    return nc


def _get_programs():
    if "l1" not in _programs:
        _programs["l1"] = _build_launch1()
    if "l2" not in _programs:
        _programs["l2"] = _build_launch2()
    return _programs["l1"], _programs["l2"]


def _expert_ffn_host(toks, w1e, b1e, w2e, b2e):
    """Exact host fallback for capacity overflow (rare)."""
    from scipy.special import erf
    h = toks @ w1e + b1e
    h = 0.5 * h * (1.0 + erf(h / np.float32(np.sqrt(2.0))))
    return h.astype(np.float32) @ w2e + b2e


def kernel(**inputs):
    import ml_dtypes

    l1, l2 = _get_programs()

    x = np.ascontiguousarray(np.asarray(inputs["x"], dtype=np.float32))        # (S,B,E)
    in_w = np.asarray(inputs["in_proj_w"], dtype=np.float32)                   # (3E,E)
    in_b = np.asarray(inputs["in_proj_b"], dtype=np.float32)
    out_w = np.asarray(inputs["out_proj_w"], dtype=np.float32)
    out_b = np.asarray(inputs["out_proj_b"], dtype=np.float32)
    gate_w = np.asarray(inputs["gate_w"], dtype=np.float32)                    # (NE,E)
    w1 = np.asarray(inputs["w1"], dtype=np.float32)                            # (NE,E,F)
    b1 = np.asarray(inputs["b1"], dtype=np.float32)
    w2 = np.asarray(inputs["w2"], dtype=np.float32)                            # (NE,F,E)
    b2 = np.asarray(inputs["b2"], dtype=np.float32)
    ln1_g = np.asarray(inputs["ln1_g"], dtype=np.float32)
    ln1_b = np.asarray(inputs["ln1_b"], dtype=np.float32)
    ln2_g = np.asarray(inputs["ln2_g"], dtype=np.float32)
    ln2_b = np.asarray(inputs["ln2_b"], dtype=np.float32)

    wqkvT = np.ascontiguousarray(in_w.T)       # (E, 3E)
    woT = np.ascontiguousarray(out_w.T)        # (E, E)
    col = lambda v: np.ascontiguousarray(v.reshape(-1, 1))

    # ---- launch 1 ----
    xT_b = [np.ascontiguousarray(x[:, b, :].T) for b in range(B)]  # (E, S) per batch
    in_maps1 = []
    for c in range(NCORES):
        b, half = divmod(c, 2)
        xb = xT_b[b]
        perm_cols = np.concatenate([
            np.arange(half * Q, half * Q + Q),
            np.arange(Q, S) if half == 0 else np.arange(0, Q),
        ])
        in_maps1.append({
            "xT": np.ascontiguousarray(xb[:, perm_cols]),
            "wqkvT": wqkvT, "bqkv": col(in_b),
            "woT": woT, "bo": col(out_b),
            "g1": col(ln1_g), "b1": col(ln1_b),
            "g2": col(ln2_g), "b2": col(ln2_b),
        })
    res1 = run_bass_kernel_spmd(l1, in_maps1, list(range(NCORES)))

    x2_all = np.empty((E, S, B), dtype=np.float32)
    h2_all = np.empty((E, S, B), dtype=np.float32)
    for c in range(NCORES):
        b, half = divmod(c, 2)
        sl = slice(half * Q, half * Q + Q)
        x2_all[:, sl, b] = res1.results[c]["x2T"]
        h2_all[:, sl, b] = res1.results[c]["h2T"]
    x2_flat = x2_all.reshape(E, N)      # token n = s*B + b
    h2_flat = h2_all.reshape(E, N)

    # ---- host gating: softmax over NE logits, top-2 renormalized ----
    logits = gate_w @ h2_flat                        # (NE, N)
    logits -= logits.max(axis=0, keepdims=True)
    p = np.exp(logits)
    p /= p.sum(axis=0, keepdims=True)
    ar = np.arange(N)
    i1 = np.argmax(p, axis=0)
    v1 = p[i1, ar]
    pm = p.copy()
    pm[i1, ar] = -1.0
    i2 = np.argmax(pm, axis=0)
    v2 = p[i2, ar]
    gsum = v1 + v2
    gate1 = v1 / gsum
    gate2 = v2 / gsum

    idx_list, gates_list, ov_list = [], [], []
    in_maps2 = []
    for e in range(NE):
        sel = np.where((i1 == e) | (i2 == e))[0]
        ge = np.where(i1[sel] == e, gate1[sel], gate2[sel]).astype(np.float32)
        ov = None
        if len(sel) > C:
            ov = (sel[C:], ge[C:])
            sel, ge = sel[:C], ge[:C]
        idx_list.append(sel)
        gates_list.append(ge)
        ov_list.append(ov)
        toksT = np.zeros((E, C), dtype=np.float32)
        toksT[:, :len(sel)] = h2_flat[:, sel]
        in_maps2.append({
            "toksT": toksT,
            "w1": w1[e],
            "w2": w2[e].astype(ml_dtypes.bfloat16),
            "b1": col(b1[e]),
            "b2": col(b2[e]),
        })
    res2 = run_bass_kernel_spmd(l2, in_maps2, list(range(NCORES)))

    # ---- combine ----
    out_flat = x2_flat
    for e in range(NE):
        sel, ge = idx_list[e], gates_list[e]
        out_flat[:, sel] += res2.results[e]["outT"][:, :len(sel)] * ge[None, :]
        if ov_list[e] is not None:
            osel, oge = ov_list[e]
            oo = _expert_ffn_host(h2_flat[:, osel].T, w1[e], b1[e], w2[e], b2[e])
            out_flat[:, osel] += oo.T * oge[None, :]

    return np.ascontiguousarray(
        out_flat.reshape(E, S, B).transpose(1, 2, 0)).astype(np.float32)


# revision 26
# speedup vs baseline: 1.2598x; 1.2598x over previous
"""MoE transformer layer on 8 Trainium2 NeuronCores.

Strategy:
  Launch 1 (attention block): shard by (batch, seq-half) -> 8 cores.
    Each core holds all 1024 tokens of its batch (for K/V) with its own
    512 query tokens ordered first, computes LN1 -> MHA -> residual ->
    LN2 entirely in a transposed [E, token] layout (E on partitions, so
    every bias/LN-gain is a per-partition scalar and no transposes are
    needed anywhere). Outputs x2T and h2T per core.
  Host: top-2 gating (softmax over 8 logits, renormalized), builds the
    per-expert token batches (all-to-all dispatch done on host).
  Launch 2 (expert FFN): expert-parallel, core e owns expert e.
    toksT [E, C] -> gelu(w1.T @ toks + b1) -> w2.T @ h + b2 -> outT.
  Host: scatter-add combine with gate weights + residual.
"""

import numpy as np

import concourse.bass as bass
import concourse.tile as tile
from concourse import bacc, mybir
from concourse.bass_utils import run_bass_kernel_spmd

S, B, E = 1024, 4, 1024
H, DH = 16, 64
F, NE = 4096, 8
N = S * B
NCORES = 8
Q = 512          # query tokens per core
KV = 1024        # key/value tokens per core (full batch-b sequence)
C = 1280         # expert capacity (max expert load for seed-0 inputs is 1076)
CT = [(0, 512), (512, 512), (1024, 256)]  # (offset, width) token tiles in launch 2
ET = E // 128    # 8
FT = F // 128    # 32

f32 = mybir.dt.float32
f32r = mybir.dt.float32r
bf16 = mybir.dt.bfloat16
AF = mybir.ActivationFunctionType
ALU = mybir.AluOpType

_GELU = AF.Gelu  # patchable for CoreSim (which lacks Gelu)

_programs = {}


def _bcast_dram(ap2d, nparts):
    """Partition-broadcast DMA source: read a [D,1] dram slice into [nparts, D]."""
    return bass.AP(tensor=ap2d.tensor, offset=ap2d.offset, ap=[[0, nparts]] + ap2d.ap)


def _build_launch1():
    nc = bacc.Bacc("TRN2", target_bir_lowering=False, debug=False, num_devices=NCORES)

    xT_d = nc.dram_tensor("xT", [E, KV], f32, kind="ExternalInput").ap()
    wqkvT_d = nc.dram_tensor("wqkvT", [E, 3 * E], f32, kind="ExternalInput").ap()
    bqkv_d = nc.dram_tensor("bqkv", [3 * E, 1], f32, kind="ExternalInput").ap()
    woT_d = nc.dram_tensor("woT", [E, E], f32, kind="ExternalInput").ap()
    bo_d = nc.dram_tensor("bo", [E, 1], f32, kind="ExternalInput").ap()
    g1_d = nc.dram_tensor("g1", [E, 1], f32, kind="ExternalInput").ap()
    b1_d = nc.dram_tensor("b1", [E, 1], f32, kind="ExternalInput").ap()
    g2_d = nc.dram_tensor("g2", [E, 1], f32, kind="ExternalInput").ap()
    b2_d = nc.dram_tensor("b2", [E, 1], f32, kind="ExternalInput").ap()
    sel_d = nc.dram_tensor("sel", [4 * 8, 128], f32, kind="ExternalInput").ap()
    x2T_d = nc.dram_tensor("x2T", [E, Q], f32, kind="ExternalOutput").ap()
    h2T_d = nc.dram_tensor("h2T", [E, Q], f32, kind="ExternalOutput").ap()

    tc_ctx = tile.TileContext(nc)
    with tc_ctx as tc:
        consts = tc.alloc_tile_pool(name="consts", bufs=1)
        statp = tc.alloc_tile_pool(name="stat", bufs=1)
        bcp = tc.alloc_tile_pool(name="bc", bufs=1)
        sqp = tc.alloc_tile_pool(name="sqp", bufs=2)
        wsp = tc.alloc_tile_pool(name="wstream", bufs=4)
        otp = tc.alloc_tile_pool(name="otp", bufs=1)
        outp = tc.alloc_tile_pool(name="outp", bufs=1)
        pmm = tc.alloc_tile_pool(name="pmm", bufs=4, space="PSUM")
        pav = tc.alloc_tile_pool(name="pav", bufs=2, space="PSUM")
        pst = tc.alloc_tile_pool(name="pst", bufs=2, space="PSUM")

        ones128 = consts.tile([128, 1], f32r, tag="ones128")
        nc.vector.memset(ones128[:].bitcast(f32), 1.0)
        ones1 = consts.tile([1, 128], f32r, tag="ones1")
        nc.vector.memset(ones1[:].bitcast(f32), 1.0)
        eps = consts.tile([1, 1], f32, tag="eps")
        nc.vector.memset(eps[:], 1e-5)

        # head-pair selector matrices (host-supplied): sel[d4].T @ recipA
        # broadcasts head-row 2*d4 to partitions 0..63, 2*d4+1 to 64..127
        sel_tiles = []
        for d4 in range(4):
            st = consts.tile([8, 128], f32r, tag=f"sel{d4}", name=f"sel{d4}")
            nc.sync.dma_start(out=st[:],
                              in_=sel_d[d4 * 8:(d4 + 1) * 8, :].bitcast(f32r))
            sel_tiles.append(st)

        def ppar(dram, k, tag):
            t = consts.tile([128, k], f32, tag=tag, name=tag)
            nc.sync.dma_start(out=t[:], in_=dram.rearrange("(a p) o -> p (a o)", p=128))
            return t

        g1_sb = ppar(g1_d, ET, "g1c")
        b1_sb = ppar(b1_d, ET, "b1c")
        g2_sb = ppar(g2_d, ET, "g2c")
        b2_sb = ppar(b2_d, ET, "b2c")
        bo_sb = ppar(bo_d, ET, "boc")
        bqkv_sb = ppar(bqkv_d, 24, "bqkvc")

        # ---------- LN helper: stats along partitions via ones-matmul ----------
        def ln_stats(src_tiles, ncols, tagpfx):
            s1 = statp.tile([1, KV], f32r, tag="s1row", name=f"{tagpfx}_s1")
            s2 = statp.tile([1, KV], f32r, tag="s2row", name=f"{tagpfx}_s2")
            tmp = statp.tile([1, KV], f32r, tag="tmprow", name=f"{tagpfx}_tmp")
            for h in range(ncols // 512):
                cs = slice(h * 512, (h + 1) * 512)
                p1 = pst.tile([1, 512], f32, tag="st", name=f"{tagpfx}_p1_{h}")
                for i in range(ET):
                    nc.tensor.matmul(p1[:], ones128[:],
                                     src_tiles[i][:, cs],
                                     start=(i == 0), stop=(i == ET - 1))
                nc.vector.tensor_copy(out=s1[:, cs], in_=p1[:])
                p2 = pst.tile([1, 512], f32, tag="st", name=f"{tagpfx}_p2_{h}")
                for i in range(ET):
                    sq = sqp.tile([128, 512], f32r, tag="sq", name=f"{tagpfx}_sq_{h}_{i}")
                    nc.vector.tensor_mul(sq[:], src_tiles[i][:, cs], src_tiles[i][:, cs])
                    nc.tensor.matmul(p2[:], ones128[:], sq[:],
                                     start=(i == 0), stop=(i == ET - 1))
                nc.vector.tensor_copy(out=s2[:, cs], in_=p2[:])
            cs = slice(0, ncols)
            # s1 <- mean ; s2 <- E[x^2] ; tmp <- mean^2 ; s2 <- var
            nc.vector.tensor_scalar(out=s1[:, cs], in0=s1[:, cs], scalar1=1.0 / E,
                                    scalar2=None, op0=ALU.mult)
            nc.vector.tensor_scalar(out=s2[:, cs], in0=s2[:, cs], scalar1=1.0 / E,
                                    scalar2=None, op0=ALU.mult)
            nc.vector.tensor_mul(tmp[:, cs], s1[:, cs], s1[:, cs])
            nc.vector.tensor_sub(s2[:, cs], s2[:, cs], tmp[:, cs])
            # s2 <- rstd = exp(-0.5*ln(var+eps))
            nc.scalar.activation(out=tmp[:, cs], in_=s2[:, cs], func=AF.Ln,
                                 bias=eps[:], scale=1.0)
            nc.scalar.activation(out=s2[:, cs], in_=tmp[:, cs], func=AF.Exp, scale=-0.5)
            # tmp <- -mean  (apply order: y = (x - mean) * rstd, then gain/bias on ACT)
            nc.vector.tensor_scalar(out=tmp[:, cs], in0=s1[:, cs], scalar1=-1.0,
                                    scalar2=None, op0=ALU.mult)
            return s2, tmp

        def bcast_rows(rowap, ncols, tagname):
            dst = bcp.tile([128, ncols], f32, tag=tagname, name=f"bc_{tagname}")
            for h in range(ncols // 512):
                cs = slice(h * 512, (h + 1) * 512)
                pb = pmm.tile([128, 512], f32, tag="mm", name=f"bc_{tagname}_{h}")
                nc.tensor.matmul(pb[:], ones1[:], rowap[:, cs],
                                 start=True, stop=True)
                nc.vector.tensor_copy(out=dst[:, cs], in_=pb[:])
            return dst

        # ---------- phase 1: load x, LN1 ----------
        xqp = tc.alloc_tile_pool(name="xqp", bufs=1)
        lxp = tc.alloc_tile_pool(name="lxp", bufs=1)
        xp = tc.alloc_tile_pool(name="xp", bufs=1)

        x_sb = []
        for i in range(ET):
            t = xp.tile([128, KV], f32r, tag=f"x{i}", name=f"x_sb{i}")
            nc.sync.dma_start(out=t[:], in_=xT_d[i * 128:(i + 1) * 128, :].bitcast(f32r))
            x_sb.append(t)

        rstd1, beta1 = ln_stats(x_sb, KV, "ln1")
        aB1 = bcast_rows(rstd1, KV, "aB1")
        bB1 = bcast_rows(beta1, KV, "bB1")

        lx = []
        xq = []
        for i in range(ET):
            t = lxp.tile([128, KV], f32r, tag=f"lx{i}", name=f"lx{i}")
            nc.vector.tensor_add(t[:], x_sb[i][:], bB1[:])
            nc.vector.tensor_mul(t[:], t[:], aB1[:])
            nc.scalar.activation(out=t[:], in_=t[:], func=AF.Identity,
                                 scale=g1_sb[:, i:i + 1], bias=b1_sb[:, i:i + 1])
            lx.append(t)
            tq = xqp.tile([128, Q], f32, tag=f"xq{i}", name=f"xq{i}")
            nc.vector.tensor_copy(out=tq[:], in_=x_sb[i][:, 0:Q])
            xq.append(tq)
        xp.release()

        # ---------- phase 2: attention ----------
        wvp = tc.alloc_tile_pool(name="wvp", bufs=1)
        vp = tc.alloc_tile_pool(name="vp", bufs=1)
        qkp = tc.alloc_tile_pool(name="qkp", bufs=2)
        attnp = tc.alloc_tile_pool(name="attnp", bufs=3)

        oT = []
        for i in range(ET):
            oT.append(otp.tile([128, Q], f32r, tag=f"oT{i}", name=f"oT{i}"))

        for half in range(2):
            # V projection for this half (8 heads), token-major with ones column
            wv = []
            for kt in range(ET):
                wt = wvp.tile([128, 512], f32r, tag=f"wv{kt}", name=f"wv_{half}_{kt}")
                nc.sync.dma_start(
                    out=wt[:],
                    in_=wqkvT_d[kt * 128:(kt + 1) * 128,
                                2 * E + half * 512: 2 * E + (half + 1) * 512].bitcast(f32r))
                wv.append(wt)
            bvB = bcp.tile([128, 512], f32, tag="bvB", name=f"bvB_{half}")
            nc.sync.dma_start(
                out=bvB[:],
                in_=_bcast_dram(bqkv_d[2 * E + half * 512: 2 * E + (half + 1) * 512, :], 128))
            v_sb = []
            for tt in range(ET):
                pv = pmm.tile([128, 512], f32, tag="mm", name=f"pv_{half}_{tt}")
                for kt in range(ET):
                    nc.tensor.matmul(pv[:],
                                     lx[kt][:, tt * 128:(tt + 1) * 128],
                                     wv[kt][:],
                                     start=(kt == 0), stop=(kt == ET - 1))
                vt = vp.tile([128, 8 * 65], f32r, tag=f"v{tt}", name=f"v_{half}_{tt}")
                nc.vector.tensor_add(
                    vt[:].rearrange("p (h d) -> p h d", h=8)[:, :, 0:64],
                    pv[:].rearrange("p (h d) -> p h d", h=8),
                    bvB[:].rearrange("p (h d) -> p h d", h=8))
                nc.vector.memset(vt[:].rearrange("p (h d) -> p h d", h=8)[:, :, 64:65].bitcast(f32), 1.0)
                v_sb.append(vt)

            denAll = statp.tile([8, Q], f32, tag="den", name=f"den_{half}", bufs=2)

            for dt in range(half * 4, half * 4 + 4):
                # qT [128, Q] for d-rows dt*128..
                pq = pmm.tile([128, Q], f32, tag="mm", name=f"pq_{dt}")
                for kt in range(ET):
                    wt = wsp.tile([128, 128], f32r, tag="wq", name=f"wq_{dt}_{kt}")
                    nc.sync.dma_start(
                        out=wt[:],
                        in_=wqkvT_d[kt * 128:(kt + 1) * 128, dt * 128:(dt + 1) * 128].bitcast(f32r))
                    nc.tensor.matmul(pq[:], wt[:],
                                     lx[kt][:, 0:Q],
                                     start=(kt == 0), stop=(kt == ET - 1))
                qT = qkp.tile([128, Q], f32r, tag="qT", name=f"qT_{dt}")
                nc.scalar.activation(out=qT[:], in_=pq[:], func=AF.Identity,
                                     bias=bqkv_sb[:, dt:dt + 1], scale=1.0)
                # kT [128, KV]
                kT = qkp.tile([128, KV], f32r, tag="kT", name=f"kT_{dt}")
                pk = [pmm.tile([128, 512], f32, tag="mm", name=f"pk_{dt}_{h}")
                      for h in range(2)]
                for kt in range(ET):
                    wt = wsp.tile([128, 128], f32r, tag="wk", name=f"wk_{dt}_{kt}")
                    nc.sync.dma_start(
                        out=wt[:],
                        in_=wqkvT_d[kt * 128:(kt + 1) * 128,
                                    E + dt * 128: E + (dt + 1) * 128].bitcast(f32r))
                    for h in range(2):
                        nc.tensor.matmul(pk[h][:], wt[:],
                                         lx[kt][:, h * 512:(h + 1) * 512],
                                         start=(kt == 0), stop=(kt == ET - 1))
                for h in range(2):
                    nc.scalar.activation(out=kT[:, h * 512:(h + 1) * 512], in_=pk[h][:],
                                         func=AF.Identity, bias=bqkv_sb[:, 8 + dt:9 + dt],
                                         scale=1.0)

                for hh in range(2):
                    hsub = slice(hh * 64, hh * 64 + 64)
                    hloc = (dt - half * 4) * 2 + hh
                    pav_t = pav.tile([65, Q], f32, tag="av", name=f"pav_{dt}_{hh}")
                    for tt in range(ET):
                        psc = pmm.tile([128, Q], f32, tag="mm", name=f"psc_{dt}_{hh}_{tt}")
                        nc.tensor.matmul(psc[:],
                                         kT[hsub, tt * 128:(tt + 1) * 128],
                                         qT[hsub, :],
                                         start=True, stop=True)
                        at = attnp.tile([128, Q], f32r, tag="attn",
                                        name=f"attn_{dt}_{hh}_{tt}")
                        nc.scalar.activation(out=at[:], in_=psc[:], func=AF.Exp,
                                             scale=0.125)
                        nc.tensor.matmul(
                            pav_t[:],
                            v_sb[tt][:].rearrange("p (h d) -> p h d", h=8)[:, hloc, :],
                            at[:],
                            start=(tt == 0), stop=(tt == ET - 1))
                    # stash unnormalized o and the softmax denominator; normalize
                    # per-half below so the slow reciprocal runs once, off the
                    # per-head critical path
                    nc.vector.tensor_copy(out=oT[dt][hsub, :], in_=pav_t[0:64, :])
                    dtmp = attnp.tile([1, Q], f32, tag="dtmp", name=f"dtmp_{dt}_{hh}",
                                      bufs=3)
                    nc.vector.tensor_copy(out=dtmp[:], in_=pav_t[64:65, :])
                    nc.gpsimd.dma_start(out=denAll[hloc:hloc + 1, :], in_=dtmp[:])

            recipA = statp.tile([8, Q], f32r, tag="recipA", name=f"recipA_{half}",
                                bufs=2)
            with nc.allow_low_precision(reason="f32r keeps fp32 bit layout"):
                nc.vector.reciprocal(recipA[:], denAll[:])
            for dt in range(half * 4, half * 4 + 4):
                prb = pmm.tile([128, Q], f32, tag="mm", name=f"prb_{dt}")
                nc.tensor.matmul(prb[:], sel_tiles[dt % 4][:], recipA[:],
                                 start=True, stop=True)
                rB = attnp.tile([128, Q], f32, tag="rB", name=f"rB_{dt}", bufs=2)
                nc.vector.tensor_copy(out=rB[:], in_=prb[:])
                nc.vector.tensor_mul(oT[dt][:, :], oT[dt][:, :], rB[:])

        attnp.release()
        qkp.release()
        vp.release()
        wvp.release()
        lxp.release()

        # ---------- phase 3: out projection + residual -> x2T ----------
        x2 = []
        for et in range(ET):
            po = pmm.tile([128, Q], f32, tag="mm", name=f"po_{et}")
            for dt in range(ET):
                wt = wsp.tile([128, 128], f32r, tag="wo", name=f"wo_{et}_{dt}")
                nc.sync.dma_start(
                    out=wt[:],
                    in_=woT_d[dt * 128:(dt + 1) * 128, et * 128:(et + 1) * 128].bitcast(f32r))
                nc.tensor.matmul(po[:], wt[:], oT[dt][:],
                                 start=(dt == 0), stop=(dt == ET - 1))
            xt = outp.tile([128, Q], f32r, tag=f"x2_{et}", name=f"x2_{et}")
            nc.scalar.activation(out=xt[:], in_=po[:], func=AF.Identity,
                                 bias=bo_sb[:, et:et + 1], scale=1.0)
            nc.vector.tensor_add(xt[:], xt[:], xq[et][:])
            nc.sync.dma_start(out=x2T_d[et * 128:(et + 1) * 128, :], in_=xt[:].bitcast(f32))
            x2.append(xt)

        # ---------- phase 4: LN2 -> h2T ----------
        rstd2, beta2 = ln_stats(x2, Q, "ln2")
        aB2 = bcast_rows(rstd2, Q, "aB2")
        bB2 = bcast_rows(beta2, Q, "bB2")
        for et in range(ET):
            t = outp.tile([128, Q], f32, tag="h2", name=f"h2_{et}", bufs=2)
            nc.vector.tensor_add(t[:], x2[et][:].bitcast(f32), bB2[:])
            nc.vector.tensor_mul(t[:], t[:], aB2[:])
            nc.scalar.activation(out=t[:], in_=t[:], func=AF.Identity,
                                 scale=g2_sb[:, et:et + 1], bias=b2_sb[:, et:et + 1])
            nc.gpsimd.dma_start(out=h2T_d[et * 128:(et + 1) * 128, :], in_=t[:])

        # release remaining pools in LIFO order per space
        xqp.release()
        outp.release()
        otp.release()
        wsp.release()
        sqp.release()
        bcp.release()
        statp.release()
        consts.release()
        pst.release()
        pav.release()
        pmm.release()

    nc.compile()
    return nc


def _build_launch2():
    nc = bacc.Bacc("TRN2", target_bir_lowering=False, debug=False, num_devices=NCORES)

    toksT_d = nc.dram_tensor("toksT", [E, C], f32, kind="ExternalInput").ap()
    w1_d = nc.dram_tensor("w1", [E, F], f32, kind="ExternalInput").ap()
    w2_d = nc.dram_tensor("w2", [F, E], bf16, kind="ExternalInput").ap()
    b1_d = nc.dram_tensor("b1", [F, 1], f32, kind="ExternalInput").ap()
    b2_d = nc.dram_tensor("b2", [E, 1], f32, kind="ExternalInput").ap()
    outT_d = nc.dram_tensor("outT", [E, C], f32, kind="ExternalOutput").ap()

    with tile.TileContext(nc) as tc:
        with (
            tc.tile_pool(name="consts", bufs=1) as consts,
            tc.tile_pool(name="tok", bufs=1) as tokp,
            tc.tile_pool(name="hp", bufs=1) as hp,
            tc.tile_pool(name="ws", bufs=6) as wsp,
            tc.tile_pool(name="outs", bufs=3) as outs,
            tc.tile_pool(name="pg1", bufs=4, space="PSUM") as pg1,
            tc.tile_pool(name="pg2", bufs=4, space="PSUM") as pg2,
        ):
            b1_sb = consts.tile([128, FT], f32, tag="b1")
            nc.sync.dma_start(out=b1_sb[:], in_=b1_d.rearrange("(a p) o -> p (a o)", p=128))
            b2_sb = consts.tile([128, ET], f32, tag="b2")
            nc.sync.dma_start(out=b2_sb[:], in_=b2_d.rearrange("(a p) o -> p (a o)", p=128))

            toks = []
            for i in range(ET):
                t = tokp.tile([128, C], f32r, tag=f"t{i}", name=f"toks{i}")
                nc.sync.dma_start(out=t[:], in_=toksT_d[i * 128:(i + 1) * 128, :].bitcast(f32r))
                toks.append(t)

            hbf = []
            for ft in range(FT):
                hbf.append(hp.tile([128, C], bf16, tag=f"h{ft}", name=f"hbf{ft}"))

            # GEMM1: hT = gelu(w1.T @ toksT + b1)
            # weight blocks [128, 256] cover two ft tiles -> half the DMA count
            for ftp in range(FT // 2):
                blks = []
                for kt in range(ET):
                    wt = wsp.tile([128, 256], f32r, tag="w1", name=f"w1_{ftp}_{kt}",
                                  bufs=12)
                    eng = nc.sync if kt % 2 == 0 else nc.gpsimd
                    eng.dma_start(
                        out=wt[:],
                        in_=w1_d[kt * 128:(kt + 1) * 128,
                                 ftp * 256:(ftp + 1) * 256].bitcast(f32r))
                    blks.append(wt)
                for sub in range(2):
                    ft = ftp * 2 + sub
                    ps = [pg1.tile([128, w], f32, tag="g1", name=f"pg1_{ft}_{ci}")
                          for ci, (off, w) in enumerate(CT)]
                    for kt in range(ET):
                        wv = blks[kt][:, sub * 128:(sub + 1) * 128]
                        for ci, (off, w) in enumerate(CT):
                            nc.tensor.matmul(ps[ci][:], wv,
                                             toks[kt][:, off:off + w],
                                             start=(kt == 0), stop=(kt == ET - 1))
                    for ci, (off, w) in enumerate(CT):
                        nc.scalar.activation(out=hbf[ft][:, off:off + w], in_=ps[ci][:],
                                             func=_GELU, bias=b1_sb[:, ft:ft + 1],
                                             scale=1.0)

            # GEMM2: outT = w2.T @ hT + b2
            # weight blocks [128, 256] cover two et tiles, kept resident across
            # both et accumulations
            for etp in range(ET // 2):
                blks = []
                for ft in range(FT):
                    wt = wsp.tile([128, 256], bf16, tag="w2", name=f"w2_{etp}_{ft}",
                                  bufs=36)
                    eng = nc.sync if ft % 2 == 0 else nc.gpsimd
                    eng.dma_start(
                        out=wt[:],
                        in_=w2_d[ft * 128:(ft + 1) * 128, etp * 256:(etp + 1) * 256])
                    blks.append(wt)
                for sub in range(2):
                    et = etp * 2 + sub
                    ps = [pg2.tile([128, w], f32, tag="g2", name=f"pg2_{et}_{ci}")
                          for ci, (off, w) in enumerate(CT)]
                    for ft in range(FT):
                        wv = blks[ft][:, sub * 128:(sub + 1) * 128]
                        for ci, (off, w) in enumerate(CT):
                            nc.tensor.matmul(ps[ci][:], wv, hbf[ft][:, off:off + w],
                                             start=(ft == 0), stop=(ft == FT - 1))
                    for ci, (off, w) in enumerate(CT):
                        ot = outs.tile([128, 512], f32, tag="ot", name=f"ot_{et}_{ci}")
                        nc.vector.tensor_scalar(out=ot[:, 0:w], in0=ps[ci][:],
                                                scalar1=b2_sb[:, et:et + 1],
                                                scalar2=None, op0=ALU.add)
                        nc.gpsimd.dma_start(
                            out=outT_d[et * 128:(et + 1) * 128, off:off + w],
                            in_=ot[:, 0:w])

    nc.compile()
    return nc


def _get_programs():
    if "l1" not in _programs:
        _programs["l1"] = _build_launch1()
    if "l2" not in _programs:
        _programs["l2"] = _build_launch2()
    return _programs["l1"], _programs["l2"]


def _expert_ffn_host(toks, w1e, b1e, w2e, b2e):
    """Exact host fallback for capacity overflow (rare)."""
    from scipy.special import erf
    h = toks @ w1e + b1e
    h = 0.5 * h * (1.0 + erf(h / np.float32(np.sqrt(2.0))))
    return h.astype(np.float32) @ w2e + b2e


def kernel(**inputs):
    import ml_dtypes

    l1, l2 = _get_programs()

    x = np.ascontiguousarray(np.asarray(inputs["x"], dtype=np.float32))        # (S,B,E)
    in_w = np.asarray(inputs["in_proj_w"], dtype=np.float32)                   # (3E,E)
    in_b = np.asarray(inputs["in_proj_b"], dtype=np.float32)
    out_w = np.asarray(inputs["out_proj_w"], dtype=np.float32)
    out_b = np.asarray(inputs["out_proj_b"], dtype=np.float32)
    gate_w = np.asarray(inputs["gate_w"], dtype=np.float32)                    # (NE,E)
    w1 = np.asarray(inputs["w1"], dtype=np.float32)                            # (NE,E,F)
    b1 = np.asarray(inputs["b1"], dtype=np.float32)
    w2 = np.asarray(inputs["w2"], dtype=np.float32)                            # (NE,F,E)
    b2 = np.asarray(inputs["b2"], dtype=np.float32)
    ln1_g = np.asarray(inputs["ln1_g"], dtype=np.float32)
    ln1_b = np.asarray(inputs["ln1_b"], dtype=np.float32)
    ln2_g = np.asarray(inputs["ln2_g"], dtype=np.float32)
    ln2_b = np.asarray(inputs["ln2_b"], dtype=np.float32)

    wqkvT = np.ascontiguousarray(in_w.T)       # (E, 3E)
    woT = np.ascontiguousarray(out_w.T)        # (E, E)
    col = lambda v: np.ascontiguousarray(v.reshape(-1, 1))

    sel = np.zeros((4 * 8, 128), dtype=np.float32)
    for d4 in range(4):
        sel[d4 * 8 + 2 * d4, 0:64] = 1.0
        sel[d4 * 8 + 2 * d4 + 1, 64:128] = 1.0

    # ---- launch 1 ----
    xT_b = [np.ascontiguousarray(x[:, b, :].T) for b in range(B)]  # (E, S) per batch
    in_maps1 = []
    for c in range(NCORES):
        b, half = divmod(c, 2)
        xb = xT_b[b]
        perm_cols = np.concatenate([
            np.arange(half * Q, half * Q + Q),
            np.arange(Q, S) if half == 0 else np.arange(0, Q),
        ])
        in_maps1.append({
            "xT": np.ascontiguousarray(xb[:, perm_cols]),
            "sel": sel,
            "wqkvT": wqkvT, "bqkv": col(in_b),
            "woT": woT, "bo": col(out_b),
            "g1": col(ln1_g), "b1": col(ln1_b),
            "g2": col(ln2_g), "b2": col(ln2_b),
        })
    res1 = run_bass_kernel_spmd(l1, in_maps1, list(range(NCORES)))

    x2_all = np.empty((E, S, B), dtype=np.float32)
    h2_all = np.empty((E, S, B), dtype=np.float32)
    for c in range(NCORES):
        b, half = divmod(c, 2)
        sl = slice(half * Q, half * Q + Q)
        x2_all[:, sl, b] = res1.results[c]["x2T"]
        h2_all[:, sl, b] = res1.results[c]["h2T"]
    x2_flat = x2_all.reshape(E, N)      # token n = s*B + b
    h2_flat = h2_all.reshape(E, N)

    # ---- host gating: softmax over NE logits, top-2 renormalized ----
    logits = gate_w @ h2_flat                        # (NE, N)
    logits -= logits.max(axis=0, keepdims=True)
    p = np.exp(logits)
    p /= p.sum(axis=0, keepdims=True)
    ar = np.arange(N)
    i1 = np.argmax(p, axis=0)
    v1 = p[i1, ar]
    pm = p.copy()
    pm[i1, ar] = -1.0
    i2 = np.argmax(pm, axis=0)
    v2 = p[i2, ar]
    gsum = v1 + v2
    gate1 = v1 / gsum
    gate2 = v2 / gsum

    idx_list, gates_list, ov_list = [], [], []
    in_maps2 = []
    for e in range(NE):
        sel = np.where((i1 == e) | (i2 == e))[0]
        ge = np.where(i1[sel] == e, gate1[sel], gate2[sel]).astype(np.float32)
        ov = None
        if len(sel) > C:
            ov = (sel[C:], ge[C:])
            sel, ge = sel[:C], ge[:C]
        idx_list.append(sel)
        gates_list.append(ge)
        ov_list.append(ov)
        toksT = np.zeros((E, C), dtype=np.float32)
        toksT[:, :len(sel)] = h2_flat[:, sel]
        in_maps2.append({
            "toksT": toksT,
            "w1": w1[e],
            "w2": w2[e].astype(ml_dtypes.bfloat16),
            "b1": col(b1[e]),
            "b2": col(b2[e]),
        })
    res2 = run_bass_kernel_spmd(l2, in_maps2, list(range(NCORES)))

    # ---- combine ----
    out_flat = x2_flat
    for e in range(NE):
        sel, ge = idx_list[e], gates_list[e]
        out_flat[:, sel] += res2.results[e]["outT"][:, :len(sel)] * ge[None, :]
        if ov_list[e] is not None:
            osel, oge = ov_list[e]
            oo = _expert_ffn_host(h2_flat[:, osel].T, w1[e], b1[e], w2[e], b2[e])
            out_flat[:, osel] += oo.T * oge[None, :]

    return np.ascontiguousarray(
        out_flat.reshape(E, S, B).transpose(1, 2, 0)).astype(np.float32)
